# revision 2
# baseline (speedup 1.0000x reference)
"""Trainium2 Bass kernel: GSpade node embedding (v3).

Computation (see reference):
  - bidirectional tanh-RNN (hidden 512/dir) over T=32768 tokens grouped into
    N=2048 contiguous ragged segments (sorted group ids in `masks`)
  - mean-pool hidden states per segment -> pooled [N, 1024]
  - out = [x @ Wx.T + bx | pooled]  -> [N, 2048]

Sharding (8 NeuronCores, SPMD single program): groups sorted by length
(desc), striped 8 ways; core c owns rank-(8i+c) segments (256 lanes) and runs
two independent scan streams (A = forward, B = backward) over them, plus 256
rows of the x-projection (bias added on host).

Scan layout: h transposed [feature(4x128p), lane]; lanes END-aligned to the
shared schedule L_i = len(rank 8i) (zero-prefix keeps h exactly 0 before a
lane's first real token: tokens zero, bias masked).  nt(t) = #{L_i > t} is a
compile-time constant schedule.

Per stream-step (n lanes), all matmuls fp8e4 DoubleRow (0.5 cyc/col):
  - input proj + masked bias: 4 MMs; stationary ktiles {16*W_ih.T | 16*b row};
    moving ktiles {tokens | aux} (aux row0 = mask).  ACT applies scale 1/16.
  - recurrence: 8 MMs (2 k-pair groups x 4 j-chunks).
  - tanh: one ACT instr psum -> h (fp8), scale=1/16
  - pooling acc (fp32 SBUF): DVE adds chunks 0-2, GPSIMD adds chunk 3 --
    off the critical path, engines otherwise idle.
A/B ping-pong keeps ACT (the throughput limit) saturated; stream A is
double-buffered in PSUM so only the 8 recurrence MMs sit between tanh(t) and
tanh(t+1).  The x-projection runs in [128,512] PSUM quarters (2 spare banks)
interleaved into the scan's PE slack; GPSIMD copies quarters out.
"""

import ml_dtypes
import numpy as np

import concourse.bacc as bacc
import concourse.mybir as mybir
from concourse.tile import TileContext
from concourse.bass_utils import run_bass_kernel_spmd

FP32 = mybir.dt.float32
BF16 = mybir.dt.bfloat16
FP8 = mybir.dt.float8e4
NP_FP8 = ml_dtypes.float8_e4m3
DR = mybir.MatmulPerfMode.DoubleRow
Tanh = mybir.ActivationFunctionType.Tanh

N_GROUPS = 2048
D_SEQ = 128
H = 512
HC = 4
D_PROJ = 1024
N_CORES = 8
LANES = 256
XROWS = N_GROUPS // N_CORES
WSCALE = 16.0
W8COLS = 2 * H + HC * H          # merged fp8 weights: [wihx | wh]
WXCOLS = 4 * D_PROJ + 4 * XROWS  # merged bf16 weights: [wx(4) | xT(4)]
TOK0_OFF = 2 * H                 # byte offset of token chunk0 in headA/headB

_program_cache: dict = {}


def _dma_chunks(nt, target=2048):
    """Chunk steps for token DMA; early chunks small so the scan starts early."""
    targets = [800, 1200, 2048, 2560]
    chunks = []
    t0 = 0
    cols = 0
    for t, n in enumerate(nt):
        tgt = targets[min(len(chunks), len(targets) - 1)]
        if cols > 0 and cols + n > tgt:
            chunks.append((t0, t))
            t0, cols = t, 0
        cols += n
    chunks.append((t0, len(nt)))
    return chunks


def _f8(a):
    return np.clip(np.asarray(a, np.float32), -240.0, 240.0).astype(NP_FP8)


def _build_program(nt):
    nt = list(nt)
    steps = len(nt)
    off = np.concatenate([[0], np.cumsum(nt)]).astype(int)
    S = int(off[-1])
    chunks = _dma_chunks(nt)

    nc = bacc.Bacc("TRN2", target_bir_lowering=False, debug=False,
                   num_devices=N_CORES)

    c0 = 2 * int(off[chunks[0][1]])          # chunk0 token cols (x2 for aux)
    tokA_d = nc.dram_tensor("tokA", [128, 2 * S], FP8, kind="ExternalInput")
    tokB_d = nc.dram_tensor("tokB", [128, 2 * S], FP8, kind="ExternalInput")
    headA_d = nc.dram_tensor("headA", [128, TOK0_OFF + c0], FP8, kind="ExternalInput")
    headB_d = nc.dram_tensor("headB", [128, TOK0_OFF + c0], FP8, kind="ExternalInput")
    whA_d = nc.dram_tensor("whA", [128, HC * H], FP8, kind="ExternalInput")
    whB_d = nc.dram_tensor("whB", [128, HC * H], FP8, kind="ExternalInput")
    wxx_d = nc.dram_tensor("wxx", [128, WXCOLS], BF16, kind="ExternalInput")
    invl_d = nc.dram_tensor("invl", [128, LANES], FP32, kind="ExternalInput")

    xp_d = nc.dram_tensor("xp", [XROWS, D_PROJ], FP32, kind="ExternalOutput")
    pool_d = nc.dram_tensor("pool", [2 * H, LANES], FP32, kind="ExternalOutput")

    with TileContext(nc) as tc:
        with (
            tc.tile_pool(name="sb", bufs=1) as sb,
            tc.tile_pool(name="psS", bufs=3, space="PSUM") as psS,
            tc.tile_pool(name="psX", bufs=2, space="PSUM") as psX,
        ):
            # ---- SBUF tiles ----
            head_sb = {}
            tok_sb = {}
            for s in ("A", "B"):
                head_sb[s] = sb.tile([128, TOK0_OFF + c0], FP8,
                                     tag=f"head{s}", name=f"head{s}")
                tok_sb[s] = sb.tile([128, 2 * S], FP8, tag=f"tok{s}", name=f"tok{s}")
            wh_tl = {s: sb.tile([128, HC * H], FP8, tag=f"wh{s}", name=f"wh{s}")
                     for s in ("A", "B")}
            invl_sb = sb.tile([128, LANES], FP32, tag="invl", name="invl")
            wxx_sb = sb.tile([128, WXCOLS], BF16, tag="wxx", name="wxx")
            h_sb = {s: [sb.tile([128, HC * LANES], FP8, tag=f"h{s}{p}", name=f"h{s}{p}")
                        for p in range(3)] for s in ("A", "B")}
            acc_sb = {s: sb.tile([128, HC * LANES], FP32, tag=f"acc{s}", name=f"acc{s}")
                      for s in ("A", "B")}
            xps_sb = [sb.tile([128, D_PROJ], FP32, tag=f"xps{bc}", name=f"xps{bc}")
                      for bc in range(2)]
            po_sb = sb.tile([128, 2 * HC * LANES], FP32, tag="po", name="po")
            po4 = po_sb.rearrange("p (s c n) -> p s c n", s=2, c=HC)
            warm_sb = sb.tile([128, 2], FP8, tag="warm", name="warm")

            # early PE ramp-up: tiny matmul as soon as possible so the
            # 3us p-state window elapses before the real matmuls arrive
            nc.vector.memset(warm_sb[:, :], 0.0)
            wps = psX.tile([128, 512], FP32, tag="xq", name="warmq")
            nc.tensor.matmul(wps[0:2, 0:2], warm_sb[:, :], warm_sb[:, :],
                             start=True, stop=True)

            # ---- DMA issue order: heads (weights+invl+chunk0), tok chunks,
            # wxx mid, rest ----
            for s, head_d in (("A", headA_d), ("B", headB_d)):
                nc.sync.dma_start(out=head_sb[s][:, :], in_=head_d[:, :])
            for s, wh_d in (("A", whA_d), ("B", whB_d)):
                nc.sync.dma_start(out=wh_tl[s][:, :], in_=wh_d[:, :])
            for k, (t0, t1) in enumerate(chunks):
                if k == 0:
                    continue
                a, b = 2 * int(off[t0]), 2 * int(off[t1])
                nc.sync.dma_start(out=tok_sb["A"][:, a:b], in_=tokA_d[:, a:b])
                nc.sync.dma_start(out=tok_sb["B"][:, a:b], in_=tokB_d[:, a:b])
                if k == 2:
                    nc.sync.dma_start(out=invl_sb[:, :], in_=invl_d[:, :])
                    nc.sync.dma_start(out=wxx_sb[:, :], in_=wxx_d[:, :])

            for s in ("A", "B"):
                nc.vector.memset(acc_sb[s][:, :], 0.0)

            # x-projection quarters: (bc, jh) -> psX tile, GPSIMD copy out.
            # Emitted interleaved into the scan (see loop below).
            def xp_mms(q):
                bc, jh = divmod(q, 2)
                xq = psX.tile([128, 512], FP32, tag="xq", name="xq")
                for kc in range(4):
                    xT_ap = wxx_sb[:, 4 * D_PROJ + kc * XROWS + bc * 128:
                                   4 * D_PROJ + kc * XROWS + (bc + 1) * 128]
                    wx_ap = wxx_sb[:, kc * D_PROJ + jh * 512:
                                   kc * D_PROJ + (jh + 1) * 512]
                    nc.tensor.matmul(xq[:, :], xT_ap, wx_ap,
                                     start=(kc == 0), stop=(kc == 3))
                return xq

            xq_tiles = {}

            # ---- scan ----
            hi_mark = {"A": nt[0], "B": nt[0]}
            t1_c0 = chunks[0][1]
            for t in range(steps):
                n = nt[t]
                a2 = 2 * int(off[t])
                for s in ("A", "B"):
                    hr = h_sb[s][(t + 2) % 3]
                    hw = h_sb[s][t % 3]
                    ps = psS.tile([128, HC * LANES], FP32, tag="ps", name="ps")
                    ps3 = ps.rearrange("p (c n) -> p c n", c=HC)
                    hw3 = hw.rearrange("p (c n) -> p c n", c=HC)
                    hr3 = hr.rearrange("p (c n) -> p c n", c=HC)
                    acc3 = acc_sb[s].rearrange("p (c n) -> p c n", c=HC)
                    wi3 = head_sb[s][:, 0:2 * H].rearrange(
                        "p (two f) -> p two f", two=2)
                    wh3 = wh_tl[s].rearrange("p (k f) -> p k f", k=HC)
                    src_t = (head_sb[s][:, TOK0_OFF + a2:TOK0_OFF + a2 + 2 * n]
                             if t < t1_c0 else tok_sb[s][:, a2:a2 + 2 * n])
                    rhs_tok = src_t.rearrange("p (two n) -> p two n", two=2)

                    # input proj + masked bias. Exactly one start=True (first
                    # write) and one stop=True (last write) per 2KB psum bank;
                    # chunks jc, jc+1 share a bank.
                    for jc in range(HC):
                        nc.tensor.matmul(ps3[:, jc, 0:n],
                                         wi3[:, :, jc * 128:(jc + 1) * 128],
                                         rhs_tok, start=(jc % 2 == 0),
                                         stop=(t == 0 and jc % 2 == 1),
                                         perf_mode=DR)
                    if t > 0:
                        for g in range(2):
                            rhs_h = hr3[:, 2 * g:2 * g + 2, 0:n]
                            for jc in range(HC):
                                nc.tensor.matmul(ps3[:, jc, 0:n],
                                                 wh3[:, 2 * g:2 * g + 2,
                                                     jc * 128:(jc + 1) * 128],
                                                 rhs_h, start=False,
                                                 stop=(g == 1 and jc % 2 == 1),
                                                 perf_mode=DR)
                    nc.scalar.activation(hw3[:, :, 0:n], ps3[:, :, 0:n], Tanh,
                                         scale=1.0 / WSCALE)
                    # pooling acc, off the critical path
                    nc.vector.tensor_add(acc3[:, 0:3, 0:n], acc3[:, 0:3, 0:n],
                                         hw3[:, 0:3, 0:n])
                    nc.gpsimd.tensor_add(acc3[:, 3, 0:n], acc3[:, 3, 0:n],
                                         hw3[:, 3, 0:n])
                    # progressive finalize of retired lanes; the A/B bands
                    # share one DMA (same band boundaries)
                    lo = nt[t + 1] if t + 1 < steps else 0
                    thresh = 64 if t < 22 else 8
                    if (hi_mark[s] - lo >= thresh and lo < hi_mark[s]) or \
                            (t == steps - 1 and lo < hi_mark[s]):
                        hi = hi_mark[s]
                        si = 0 if s == "A" else 1
                        nc.vector.tensor_mul(
                            po4[:, si, :, lo:hi], acc3[:, :, lo:hi],
                            invl_sb[:, lo:hi].unsqueeze(1).to_broadcast(
                                [128, HC, hi - lo]))
                        hi_mark[s] = lo
                        if s == "B":
                            dst = pool_d[:, lo:hi].rearrange(
                                "(s c p) l -> p s c l", s=2, c=HC)
                            nc.sync.dma_start(out=dst, in_=po4[:, :, :, lo:hi])

                # interleave x-projection into the scan's PE slack
                if t in (6, 8, 10, 12):
                    q = (t - 6) // 2
                    xq_tiles[q] = xp_mms(q)
                if t in (8, 10, 12, 14):
                    q = (t - 8) // 2
                    bc, jh = divmod(q, 2)
                    nc.vector.tensor_copy(
                        xps_sb[bc][:, jh * 512:(jh + 1) * 512], xq_tiles[q][:, :])
                if t == 16:
                    for bc in range(2):
                        nc.sync.dma_start(out=xp_d[bc * 128:(bc + 1) * 128, :],
                                          in_=xps_sb[bc][:, :])


    nc.compile()
    return nc


def _get_program(nt):
    key = tuple(nt)
    if key not in _program_cache:
        _program_cache[key] = _build_program(nt)
    return _program_cache[key]


def _prepare(x, seqs, masks, W_ih_f, W_hh_f, b_f, W_ih_b, W_hh_b, b_b, Wx, bx):
    x = np.asarray(x, np.float32)
    seqs = np.asarray(seqs, np.float32)
    masks = np.asarray(masks).astype(np.int64)

    lens = np.bincount(masks, minlength=N_GROUPS).astype(np.int64)
    starts_all = np.concatenate([[0], np.cumsum(lens)[:-1]])
    order = np.argsort(-lens, kind="stable")
    L = lens[order[0::N_CORES]].astype(np.int64)
    steps = int(L[0])
    nt = [int((L > t).sum()) for t in range(steps)]
    off = np.concatenate([[0], np.cumsum(nt)]).astype(int)
    S = int(off[-1])

    t_grid = np.arange(steps)[:, None]
    active = t_grid < L[None, :]
    seqs_pad = np.vstack([np.zeros((1, D_SEQ), np.float32), seqs])
    seqs_f8 = _f8(seqs_pad)

    gid = [order[c::N_CORES] for c in range(N_CORES)]
    per_core = {}
    for c in range(N_CORES):
        lens_c = lens[gid[c]]
        starts_c = starts_all[gid[c]]
        pre = (L - lens_c)[None, :]
        real = active & (t_grid >= pre)
        pos = t_grid - pre
        idx_f = np.where(real, starts_c[None, :] + pos, -1)
        idx_b = np.where(real, starts_c[None, :] + lens_c[None, :] - 1 - pos, -1)
        mask_flat = real[active].astype(np.float32)

        def stream(idx):
            tokens = seqs_f8[idx[active] + 1].T
            out = np.zeros((128, 2 * S), NP_FP8)
            for t in range(steps):
                n = nt[t]
                a = off[t]
                out[:, 2 * a:2 * a + n] = tokens[:, a:a + n]
                out[0, 2 * a + n:2 * a + 2 * n] = _f8(mask_flat[a:a + n])
            return out

        invl = np.ascontiguousarray(np.broadcast_to(
            (1.0 / lens_c).astype(np.float32)[None, :], (128, LANES)))
        per_core[c] = (stream(idx_f), stream(idx_b), invl)

    def w8(W_ih, b, W_hh):
        wi = np.zeros((128, 2, H), np.float32)
        wi[:, 0, :] = WSCALE * np.asarray(W_ih, np.float32).T
        wi[0, 1, :] = WSCALE * np.asarray(b, np.float32)
        wh = WSCALE * np.asarray(W_hh, np.float32).T
        wh = wh.reshape(HC, 128, H).transpose(1, 0, 2)
        return _f8(np.concatenate(
            [wi.reshape(128, 2 * H), wh.reshape(128, HC * H)], axis=1))

    w8A = w8(W_ih_f, b_f, W_hh_f)
    w8B = w8(W_ih_b, b_b, W_hh_b)
    chunks = _dma_chunks(nt)
    c0 = 2 * int(off[chunks[0][1]])

    wxT = np.asarray(Wx, np.float32).T          # [512, 1024]
    in_maps = []
    for c in range(N_CORES):
        tokA, tokB, invl = per_core[c]
        xT = x[c * XROWS:(c + 1) * XROWS, :].T  # [512, 256]
        wxx = np.concatenate(
            [wxT.reshape(4, 128, D_PROJ).reshape(4, 128, D_PROJ).transpose(
                1, 0, 2).reshape(128, 4 * D_PROJ),
             xT.reshape(4, 128, XROWS).transpose(1, 0, 2).reshape(
                 128, 4 * XROWS)],
            axis=1).astype(ml_dtypes.bfloat16)
        headA = np.concatenate([w8A[:, :2 * H], tokA[:, :c0]], axis=1)
        headB = np.concatenate([w8B[:, :2 * H], tokB[:, :c0]], axis=1)
        in_maps.append({
            "tokA": tokA, "tokB": tokB,
            "headA": headA, "headB": headB,
            "whA": w8A[:, 2 * H:].copy(), "whB": w8B[:, 2 * H:].copy(),
            "wxx": wxx, "invl": invl,
        })
    return nt, in_maps, gid


def _assemble(res, gid, bx):
    out = np.empty((N_GROUPS, 2 * D_PROJ), np.float32)
    for c in range(N_CORES):
        out[c * XROWS:(c + 1) * XROWS, :D_PROJ] = res[c]["xp"]
        out[gid[c], D_PROJ:] = res[c]["pool"].T
    out[:, :D_PROJ] += np.asarray(bx, np.float32)[None, :]
    return out


def kernel(**inputs):
    nt, in_maps, gid = _prepare(**inputs)
    nc = _get_program(nt)
    res = run_bass_kernel_spmd(nc, in_maps, list(range(N_CORES))).results
    return _assemble(res, gid, inputs["bx"])
